# revision 4
# baseline (speedup 1.0000x reference)
"""Trainium2 Bass kernel for a pre-norm adapter layer (LN -> down -> GELU -> up -> +residual).

Data-parallel across 8 NeuronCores: each core processes 4096 tokens of the
(8, 4096, 1024) input.  LayerNorm gamma/beta are folded into the down
projection on the host; b_up is folded into the up matmul via an augmented
ones-row on the stationary operand.

Self-contained: hardcodes shapes from the problem spec.
"""

import numpy as np

import concourse.bass as bass
import concourse.bacc as bacc
import concourse.mybir as mybir
import concourse.tile as tile
from concourse.bass_utils import run_bass_kernel_spmd
from concourse.masks import make_identity

LN_EPS = 1e-5
B, S, H, R = 8, 4096, 1024, 64
N_CORES = 8
TOK = (B * S) // N_CORES  # tokens per core = 4096
P = 128                   # partitions / tokens per tile
N_TILES = TOK // P        # 32
KSLC = H // P             # 8 contraction slices of 128

F32 = mybir.dt.float32
ADD = mybir.AluOpType.add
SUB = mybir.AluOpType.subtract
MULT = mybir.AluOpType.mult
AFT = mybir.ActivationFunctionType


def build_kernel() -> bass.Bass:
    nc = bacc.Bacc()

    x_ext = nc.declare_dram_parameter("hidden_states", [TOK, H], F32, isOutput=False)
    wd_ext = nc.declare_dram_parameter("w_down", [H, R], F32, isOutput=False)
    bd_ext = nc.declare_dram_parameter("b_down", [R, 1], F32, isOutput=False)
    wua_ext = nc.declare_dram_parameter("w_up_aug", [R + 1, H], F32, isOutput=False)
    out_ext = nc.declare_dram_parameter("out", [TOK, H], F32, isOutput=True)

    x_rows = x_ext.rearrange("(n p) h -> n p h", p=P)
    out_rows = out_ext.rearrange("(n p) h -> n p h", p=P)

    with tile.TileContext(nc) as tc:
        with (
            tc.tile_pool(name="singles", bufs=1) as singles,
            tc.tile_pool(name="xin", bufs=3) as xin_pool,
            tc.tile_pool(name="stats", bufs=4) as stats_pool,
            tc.tile_pool(name="xhat", bufs=2) as xhat_pool,
            tc.tile_pool(name="xT", bufs=2) as xT_pool,
            tc.tile_pool(name="h1g", bufs=2) as h1g_pool,
            tc.tile_pool(name="outp", bufs=3) as out_pool,
            tc.tile_pool(name="ps_t", bufs=2, space="PSUM") as ps_t,
            tc.tile_pool(name="ps_h1", bufs=2, space="PSUM") as ps_h1,
            tc.tile_pool(name="ps_o", bufs=2, space="PSUM") as ps_o,
        ):
            # --- one-time loads -------------------------------------------------
            wd_sb = singles.tile([P, KSLC, R], F32)  # [h%128, hslice, r]
            nc.sync.dma_start(out=wd_sb, in_=wd_ext.rearrange("(k p) r -> p k r", p=P))
            wua_sb = singles.tile([R + 1, H], F32)
            nc.sync.dma_start(out=wua_sb, in_=wua_ext[:])
            bd_sb = singles.tile([R, 1], F32)
            nc.sync.dma_start(out=bd_sb, in_=bd_ext[:])
            eps_sb = singles.tile([P, 1], F32)
            nc.vector.memset(eps_sb, LN_EPS)
            ident = singles.tile([P, P], F32)
            make_identity(nc, ident)

            # --- main loop over 32 token tiles ---------------------------------
            for i in range(N_TILES):
                x_sb = xin_pool.tile([P, H], F32)
                nc.sync.dma_start(out=x_sb, in_=x_rows[i])

                # LayerNorm statistics (mean/var over H)
                stats = stats_pool.tile([P, 2, 6], F32)
                nc.vector.bn_stats(stats[:, 0, :], x_sb[:, 0:512])
                nc.vector.bn_stats(stats[:, 1, :], x_sb[:, 512:1024])
                mv = stats_pool.tile([P, 2], F32)
                nc.vector.bn_aggr(mv, stats)
                rstd = stats_pool.tile([P, 1], F32)
                nc.scalar.activation(rstd, mv[:, 1:2], AFT.Sqrt, bias=eps_sb, scale=1.0)
                nc.vector.reciprocal(rstd, rstd)

                # xhat = (x - mean) * rstd
                xhat = xhat_pool.tile([P, H], F32)
                nc.vector.tensor_scalar(
                    out=xhat, in0=x_sb,
                    scalar1=mv[:, 0:1], scalar2=rstd,
                    op0=SUB, op1=MULT,
                )

                # transpose xhat -> xT  ([token, h] -> [h, token]), 8 slices
                xT = xT_pool.tile([P, H], F32)
                for half in range(2):
                    pt = ps_t.tile([P, 512], F32, tag="pt")
                    for q in range(4):
                        k = half * 4 + q
                        nc.tensor.transpose(
                            pt[:, q * P:(q + 1) * P],
                            xhat[:, k * P:(k + 1) * P],
                            ident,
                        )
                    nc.scalar.copy(out=xT[:, half * 512:(half + 1) * 512], in_=pt)

                # down-proj: h1[r, t] = sum_h wd[h, r] * xhat[t, h]
                h1 = ps_h1.tile([R, P], F32)
                for k in range(KSLC):
                    nc.tensor.matmul(
                        h1,
                        lhsT=wd_sb[:, k, :],
                        rhs=xT[:, k * P:(k + 1) * P],
                        start=(k == 0), stop=(k == KSLC - 1),
                    )

                # GELU(h1 + b_down); augment with a ones row for the up bias
                h1g = h1g_pool.tile([R + 1, P], F32)
                nc.gpsimd.memset(h1g[R:R + 1, :], 1.0)
                nc.scalar.activation(h1g[0:R, :], h1, AFT.Gelu, bias=bd_sb, scale=1.0)

                # up-proj: out[t, h] = sum_r h1g[r, t] * wua[r, h]  (+ b_up row)
                po = ps_o.tile([P, H], F32)
                nc.tensor.matmul(po[:, 0:512], lhsT=h1g, rhs=wua_sb[:, 0:512],
                                 start=True, stop=True)
                nc.tensor.matmul(po[:, 512:1024], lhsT=h1g, rhs=wua_sb[:, 512:1024],
                                 start=True, stop=True)

                # residual add + store
                o_sb = out_pool.tile([P, H], F32)
                nc.vector.tensor_tensor(out=o_sb, in0=po, in1=x_sb, op=ADD)
                nc.sync.dma_start(out=out_rows[i], in_=o_sb)

    return nc


_CACHE: dict = {}


def _get_nc() -> bass.Bass:
    if "nc" not in _CACHE:
        nc = build_kernel()
        nc.finalize()
        _CACHE["nc"] = nc
    return _CACHE["nc"]


def make_in_maps(hidden_states, ln_gamma, ln_beta, w_down, b_down, w_up, b_up):
    x = np.ascontiguousarray(np.asarray(hidden_states, dtype=np.float32))
    g = np.asarray(ln_gamma, dtype=np.float32)
    be = np.asarray(ln_beta, dtype=np.float32)
    wd = np.asarray(w_down, dtype=np.float32)
    bd = np.asarray(b_down, dtype=np.float32)
    wu = np.asarray(w_up, dtype=np.float32)
    bu = np.asarray(b_up, dtype=np.float32)

    # Fold LN affine into the down projection:
    #   (xhat*g + be) @ wd + bd == xhat @ (g[:,None]*wd) + (be @ wd + bd)
    wd_eff = np.ascontiguousarray(g[:, None] * wd)
    bd_eff = np.ascontiguousarray((bd + be @ wd).reshape(R, 1))
    # Fold b_up into the up matmul via an appended ones-row on the left operand.
    wua = np.ascontiguousarray(np.concatenate([wu, bu[None, :]], axis=0))

    x_shards = x.reshape(N_CORES, TOK, H)
    return [
        {
            "hidden_states": np.ascontiguousarray(x_shards[c]),
            "w_down": wd_eff,
            "b_down": bd_eff,
            "w_up_aug": wua,
        }
        for c in range(N_CORES)
    ]


def run_device(in_maps, **kwargs):
    nc = _get_nc()
    return run_bass_kernel_spmd(nc, in_maps, core_ids=list(range(N_CORES)), **kwargs)


def kernel(hidden_states, ln_gamma, ln_beta, w_down, b_down, w_up, b_up):
    in_maps = make_in_maps(hidden_states, ln_gamma, ln_beta,
                           w_down, b_down, w_up, b_up)
    res = run_device(in_maps)
    out = np.stack([res.results[c]["out"] for c in range(N_CORES)], axis=0)
    return np.ascontiguousarray(out.reshape(B, S, H).astype(np.float32, copy=False))


# revision 6
# speedup vs baseline: 1.0663x; 1.0663x over previous
"""Trainium2 Bass kernel for a pre-norm adapter layer (LN -> down -> GELU -> up -> +residual).

Data-parallel across 8 NeuronCores: each core processes 4096 tokens of the
(8, 4096, 1024) input.  LayerNorm gamma/beta are folded into the down
projection on the host; b_up is folded into the up matmul via an augmented
ones-row on the stationary operand.

v2: bf16 matmul path (4x PE speedup vs fp32), LN rstd via DVE-only Newton
rsqrt batched per tile-group (kills ACT table thrashing), residual add on
GpSimd, PSUM->SBUF copies load-balanced via nc.any.

Self-contained: hardcodes shapes from the problem spec.
"""

import numpy as np
import ml_dtypes

import concourse.bass as bass
import concourse.bacc as bacc
import concourse.mybir as mybir
import concourse.tile as tile
from concourse.bass_utils import run_bass_kernel_spmd
from concourse.masks import make_identity

LN_EPS = 1e-5
B, S, H, R = 8, 4096, 1024, 64
N_CORES = 8
TOK = (B * S) // N_CORES  # tokens per core = 4096
P = 128                   # partitions / tokens per tile
N_TILES = TOK // P        # 32
KSLC = H // P             # 8 contraction slices of 128
# LN-stat groups: small first groups prime the pipeline quickly, larger
# groups amortize the per-group Newton-rsqrt cost.
GROUPS = [2, 2, 4, 8, 8, 8]
assert sum(GROUPS) == N_TILES
GMAX = max(GROUPS)

F32 = mybir.dt.float32
BF16 = mybir.dt.bfloat16
I32 = mybir.dt.int32
ALU = mybir.AluOpType
AFT = mybir.ActivationFunctionType


def build_kernel() -> bass.Bass:
    nc = bacc.Bacc()

    x_ext = nc.declare_dram_parameter("hidden_states", [TOK, H], F32, isOutput=False)
    wd_ext = nc.declare_dram_parameter("w_down", [H, R], BF16, isOutput=False)
    bd_ext = nc.declare_dram_parameter("b_down", [R, 1], F32, isOutput=False)
    wua_ext = nc.declare_dram_parameter("w_up_aug", [R + 1, H], BF16, isOutput=False)
    out_ext = nc.declare_dram_parameter("out", [TOK, H], F32, isOutput=True)

    x_rows = x_ext.rearrange("(n p) h -> n p h", p=P)
    out_rows = out_ext.rearrange("(n p) h -> n p h", p=P)

    with tile.TileContext(nc) as tc:
        with (
            tc.tile_pool(name="singles", bufs=1) as singles,
            tc.tile_pool(name="xin", bufs=14) as xin_pool,
            tc.tile_pool(name="bns", bufs=3) as bns_pool,
            tc.tile_pool(name="gstat", bufs=2) as gstat_pool,
            tc.tile_pool(name="xhat", bufs=3) as xhat_pool,
            tc.tile_pool(name="xT", bufs=3) as xT_pool,
            tc.tile_pool(name="h1g", bufs=3) as h1g_pool,
            tc.tile_pool(name="outp", bufs=4) as out_pool,
            tc.tile_pool(name="ps_t", bufs=2, space="PSUM") as ps_t,
            tc.tile_pool(name="ps_h1", bufs=2, space="PSUM") as ps_h1,
            tc.tile_pool(name="ps_o", bufs=2, space="PSUM") as ps_o,
        ):
            # --- one-time loads -------------------------------------------------
            wd_sb = singles.tile([P, KSLC, R], BF16)  # [h%128, hslice, r]
            nc.sync.dma_start(out=wd_sb, in_=wd_ext.rearrange("(k p) r -> p k r", p=P))
            wua_sb = singles.tile([R + 1, H], BF16)
            nc.sync.dma_start(out=wua_sb, in_=wua_ext[:])
            bd_sb = singles.tile([R, 1], F32)
            nc.sync.dma_start(out=bd_sb, in_=bd_ext[:])
            ident = singles.tile([P, P], BF16)
            make_identity(nc, ident)

            def process_tile(i, mean_ap, rstd_ap):
                x_sb = x_tiles[i]
                # xhat = (x - mean) * rstd, cast to bf16
                xhat = xhat_pool.tile([P, H], BF16, tag="xhat")
                nc.vector.tensor_scalar(
                    out=xhat, in0=x_sb,
                    scalar1=mean_ap, scalar2=rstd_ap,
                    op0=ALU.subtract, op1=ALU.mult,
                )
                # transpose xhat -> xT ([token, h] -> [h, token]), 8 slices
                xT = xT_pool.tile([P, H], BF16, tag="xT")
                for half in range(2):
                    pt = ps_t.tile([P, 512], BF16, tag="pt")
                    for q in range(4):
                        k = half * 4 + q
                        nc.tensor.transpose(
                            pt[:, q * P:(q + 1) * P],
                            xhat[:, k * P:(k + 1) * P],
                            ident,
                        )
                    nc.any.tensor_copy(
                        out=xT[:, half * 512:(half + 1) * 512], in_=pt)

                # down-proj: h1[r, t] = sum_h wd[h, r] * xhat[t, h]
                h1 = ps_h1.tile([R, P], F32, tag="h1")
                for k in range(KSLC):
                    nc.tensor.matmul(
                        h1,
                        lhsT=wd_sb[:, k, :],
                        rhs=xT[:, k * P:(k + 1) * P],
                        start=(k == 0), stop=(k == KSLC - 1),
                    )

                # GELU(h1 + b_down); ones row folds b_up into the up matmul
                h1g = h1g_pool.tile([R + 1, P], BF16, tag="h1g")
                nc.gpsimd.memset(h1g[R:R + 1, :], 1.0)
                nc.scalar.activation(h1g[0:R, :], h1, AFT.Gelu, bias=bd_sb, scale=1.0)

                # up-proj: out[t, h] = sum_r h1g[r, t] * wua[r, h]
                po = ps_o.tile([P, H], F32, tag="po")
                nc.tensor.matmul(po[:, 0:512], lhsT=h1g, rhs=wua_sb[:, 0:512],
                                 start=True, stop=True)
                nc.tensor.matmul(po[:, 512:1024], lhsT=h1g, rhs=wua_sb[:, 512:1024],
                                 start=True, stop=True)

                # residual: o = po + x  (copy PSUM->SBUF, add on GpSimd)
                o_sb = out_pool.tile([P, H], F32, tag="o")
                nc.any.tensor_copy(out=o_sb, in_=po)
                nc.gpsimd.tensor_tensor(out=o_sb, in0=o_sb, in1=x_sb, op=ALU.add)
                nc.sync.dma_start(out=out_rows[i], in_=o_sb)

            # --- main loop: groups of tiles -----------------------------------
            x_tiles = {}
            base = 0
            for g in GROUPS:
                # load + LN stats for the group
                mvg = gstat_pool.tile([P, GMAX, 2], F32, tag="mvg")
                for j in range(g):
                    i = base + j
                    x_sb = xin_pool.tile([P, H], F32, tag="x")
                    x_tiles[i] = x_sb
                    nc.sync.dma_start(out=x_sb, in_=x_rows[i])
                    st = bns_pool.tile([P, 2, 6], F32, tag="bns")
                    nc.vector.bn_stats(st[:, 0, :], x_sb[:, 0:512])
                    nc.vector.bn_stats(st[:, 1, :], x_sb[:, 512:1024])
                    nc.vector.bn_aggr(mvg[:, j, :], st)

                # rstd for the whole group: Newton rsqrt on DVE (no ACT tables)
                vd = gstat_pool.tile([P, GMAX], F32, tag="vd")
                nc.vector.tensor_scalar(
                    out=vd[:, 0:g], in0=mvg[:, 0:g, 1],
                    scalar1=LN_EPS, scalar2=None, op0=ALU.add)
                rg = gstat_pool.tile([P, GMAX], F32, tag="rg")
                t1 = gstat_pool.tile([P, GMAX], F32, tag="t1")
                t2 = gstat_pool.tile([P, GMAX], F32, tag="t2")
                # y0 bits = 0x5f3759df - (bits(v) >> 1)
                nc.vector.tensor_scalar(
                    out=rg.bitcast(I32)[:, 0:g], in0=vd.bitcast(I32)[:, 0:g],
                    scalar1=1, scalar2=0xFFFFFFFF,
                    op0=ALU.logical_shift_right, op1=ALU.bitwise_xor)
                nc.vector.tensor_scalar(
                    out=rg.bitcast(I32)[:, 0:g], in0=rg.bitcast(I32)[:, 0:g],
                    scalar1=0x5F3759E0, scalar2=None, op0=ALU.add)
                for _ in range(3):  # y *= 1.5 - 0.5*v*y*y
                    nc.vector.tensor_mul(out=t1[:, 0:g], in0=rg[:, 0:g], in1=rg[:, 0:g])
                    nc.vector.tensor_mul(out=t2[:, 0:g], in0=t1[:, 0:g], in1=vd[:, 0:g])
                    nc.vector.tensor_scalar(
                        out=t2[:, 0:g], in0=t2[:, 0:g],
                        scalar1=-0.5, scalar2=1.5, op0=ALU.mult, op1=ALU.add)
                    nc.vector.tensor_mul(out=rg[:, 0:g], in0=rg[:, 0:g], in1=t2[:, 0:g])

                # adapter math for each tile in the group
                for j in range(g):
                    process_tile(base + j, mvg[:, j, 0:1], rg[:, j:j + 1])
                base += g

    return nc


_CACHE: dict = {}


def _get_nc() -> bass.Bass:
    if "nc" not in _CACHE:
        nc = build_kernel()
        nc.finalize()
        _CACHE["nc"] = nc
    return _CACHE["nc"]


def make_in_maps(hidden_states, ln_gamma, ln_beta, w_down, b_down, w_up, b_up):
    x = np.ascontiguousarray(np.asarray(hidden_states, dtype=np.float32))
    g = np.asarray(ln_gamma, dtype=np.float32)
    be = np.asarray(ln_beta, dtype=np.float32)
    wd = np.asarray(w_down, dtype=np.float32)
    bd = np.asarray(b_down, dtype=np.float32)
    wu = np.asarray(w_up, dtype=np.float32)
    bu = np.asarray(b_up, dtype=np.float32)

    # Fold LN affine into the down projection:
    #   (xhat*g + be) @ wd + bd == xhat @ (g[:,None]*wd) + (be @ wd + bd)
    wd_eff = np.ascontiguousarray((g[:, None] * wd).astype(ml_dtypes.bfloat16))
    bd_eff = np.ascontiguousarray((bd + be @ wd).reshape(R, 1).astype(np.float32))
    # Fold b_up into the up matmul via an appended ones-row on the left operand.
    wua = np.ascontiguousarray(
        np.concatenate([wu, bu[None, :]], axis=0).astype(ml_dtypes.bfloat16))

    x_shards = x.reshape(N_CORES, TOK, H)
    return [
        {
            "hidden_states": np.ascontiguousarray(x_shards[c]),
            "w_down": wd_eff,
            "b_down": bd_eff,
            "w_up_aug": wua,
        }
        for c in range(N_CORES)
    ]


def run_device(in_maps, **kwargs):
    nc = _get_nc()
    return run_bass_kernel_spmd(nc, in_maps, core_ids=list(range(N_CORES)), **kwargs)


def kernel(hidden_states, ln_gamma, ln_beta, w_down, b_down, w_up, b_up):
    in_maps = make_in_maps(hidden_states, ln_gamma, ln_beta,
                           w_down, b_down, w_up, b_up)
    res = run_device(in_maps)
    out = np.stack([res.results[c]["out"] for c in range(N_CORES)], axis=0)
    return np.ascontiguousarray(out.reshape(B, S, H).astype(np.float32, copy=False))


# revision 10
# speedup vs baseline: 1.5009x; 1.4075x over previous
"""Trainium2 Bass kernel for a pre-norm adapter layer (LN -> down -> GELU -> up -> +residual).

Data-parallel across 8 NeuronCores: each core processes 4096 tokens of the
(8, 4096, 1024) input.  LayerNorm gamma/beta are folded into the down
projection on the host; b_up is folded into the up matmul via an augmented
ones-row on the stationary operand.

v2: bf16 matmul path (4x PE speedup vs fp32), LN rstd via DVE-only Newton
rsqrt batched per tile-group (kills ACT table thrashing), residual add on
GpSimd, PSUM->SBUF copies load-balanced via nc.any.

Self-contained: hardcodes shapes from the problem spec.
"""

import numpy as np
import ml_dtypes

import concourse.bass as bass
import concourse.bacc as bacc
import concourse.mybir as mybir
import concourse.tile as tile
from concourse.bass_utils import run_bass_kernel_spmd
from concourse.masks import make_identity

LN_EPS = 1e-5
B, S, H, R = 8, 4096, 1024, 64
N_CORES = 8
TOK = (B * S) // N_CORES  # tokens per core = 4096
P = 128                   # partitions / tokens per tile
N_TILES = TOK // P        # 32
KSLC = H // P             # 8 contraction slices of 128
# LN-stat groups: small first groups prime the pipeline quickly, larger
# groups amortize the per-group Newton-rsqrt cost.
GROUPS = [2, 2, 4, 8, 8, 8]
assert sum(GROUPS) == N_TILES
GMAX = max(GROUPS)

F32 = mybir.dt.float32
BF16 = mybir.dt.bfloat16
I32 = mybir.dt.int32
ALU = mybir.AluOpType
AFT = mybir.ActivationFunctionType


def build_kernel() -> bass.Bass:
    nc = bacc.Bacc()

    x_ext = nc.declare_dram_parameter("hidden_states", [TOK, H], F32, isOutput=False)
    wd_ext = nc.declare_dram_parameter("w_down", [H, R], BF16, isOutput=False)
    bd_ext = nc.declare_dram_parameter("b_down", [R, 1], F32, isOutput=False)
    wua_ext = nc.declare_dram_parameter("w_up_aug", [R + 1, H], BF16, isOutput=False)
    out_ext = nc.declare_dram_parameter("out", [TOK, H], F32, isOutput=True)

    x_rows = x_ext.rearrange("(n p) h -> n p h", p=P)
    out_rows = out_ext.rearrange("(n p) h -> n p h", p=P)

    with tile.TileContext(nc) as tc:
        with (
            tc.tile_pool(name="singles", bufs=1) as singles,
            tc.tile_pool(name="xin", bufs=20) as xin_pool,
            tc.tile_pool(name="bns", bufs=3) as bns_pool,
            tc.tile_pool(name="gstat", bufs=2) as gstat_pool,
            tc.tile_pool(name="xhat", bufs=3) as xhat_pool,
            tc.tile_pool(name="xT", bufs=3) as xT_pool,
            tc.tile_pool(name="h1g", bufs=3) as h1g_pool,
            tc.tile_pool(name="outp", bufs=4) as out_pool,
            tc.tile_pool(name="ps_t", bufs=2, space="PSUM") as ps_t,
            tc.tile_pool(name="ps_h1", bufs=2, space="PSUM") as ps_h1,
            tc.tile_pool(name="ps_o", bufs=2, space="PSUM") as ps_o,
        ):
            # --- one-time loads -------------------------------------------------
            wd_sb = singles.tile([P, KSLC, R], BF16)  # [h%128, hslice, r]
            nc.sync.dma_start(out=wd_sb, in_=wd_ext.rearrange("(k p) r -> p k r", p=P))
            wua_sb = singles.tile([R + 1, H], BF16)
            nc.sync.dma_start(out=wua_sb, in_=wua_ext[:])
            bd_sb = singles.tile([R, 1], F32)
            nc.sync.dma_start(out=bd_sb, in_=bd_ext[:])
            ident = singles.tile([P, P], BF16)
            make_identity(nc, ident)

            def process_tile(i, mean_ap, rstd_ap):
                x_sb = x_tiles[i]
                # xhat = (x - mean) * rstd, cast to bf16
                xhat = xhat_pool.tile([P, H], BF16, tag="xhat")
                nc.vector.tensor_scalar(
                    out=xhat, in0=x_sb,
                    scalar1=mean_ap, scalar2=rstd_ap,
                    op0=ALU.subtract, op1=ALU.mult,
                )
                # transpose xhat -> xT ([token, h] -> [h, token]), 8 slices
                xT = xT_pool.tile([P, H], BF16, tag="xT")
                for half in range(2):
                    pt = ps_t.tile([P, 512], BF16, tag="pt")
                    for q in range(4):
                        k = half * 4 + q
                        nc.tensor.transpose(
                            pt[:, q * P:(q + 1) * P],
                            xhat[:, k * P:(k + 1) * P],
                            ident,
                        )
                    nc.scalar.copy(
                        out=xT[:, half * 512:(half + 1) * 512], in_=pt)

                # down-proj: h1[r, t] = sum_h wd[h, r] * xhat[t, h]
                h1 = ps_h1.tile([R, P], F32, tag="h1")
                for k in range(KSLC):
                    nc.tensor.matmul(
                        h1,
                        lhsT=wd_sb[:, k, :],
                        rhs=xT[:, k * P:(k + 1) * P],
                        start=(k == 0), stop=(k == KSLC - 1),
                    )

                # GELU(h1 + b_down); ones row folds b_up into the up matmul
                h1g = h1g_pool.tile([R + 1, P], BF16, tag="h1g")
                nc.gpsimd.memset(h1g[R:R + 1, :], 1.0)
                nc.scalar.activation(h1g[0:R, :], h1, AFT.Gelu, bias=bd_sb, scale=1.0)

                # up-proj: out[t, h] = sum_r h1g[r, t] * wua[r, h]
                po = ps_o.tile([P, H], F32, tag="po")
                nc.tensor.matmul(po[:, 0:512], lhsT=h1g, rhs=wua_sb[:, 0:512],
                                 start=True, stop=True)
                nc.tensor.matmul(po[:, 512:1024], lhsT=h1g, rhs=wua_sb[:, 512:1024],
                                 start=True, stop=True)

                # residual: o = po + x  (copy PSUM->SBUF on ACT, add on GpSimd)
                o_sb = out_pool.tile([P, H], F32, tag="o")
                nc.scalar.copy(out=o_sb, in_=po)
                nc.gpsimd.tensor_tensor(out=o_sb, in0=o_sb, in1=x_sb, op=ALU.add)
                nc.sync.dma_start(out=out_rows[i], in_=o_sb)

            def group_rstd(mvg, g):
                # rstd for the whole group: Newton rsqrt on DVE (no ACT tables)
                vd = gstat_pool.tile([P, GMAX], F32, tag="vd")
                nc.vector.tensor_scalar(
                    out=vd[:, 0:g], in0=mvg[:, 0:g, 1],
                    scalar1=LN_EPS, scalar2=None, op0=ALU.add)
                rg = gstat_pool.tile([P, GMAX], F32, tag="rg")
                t1 = gstat_pool.tile([P, GMAX], F32, tag="t1")
                t2 = gstat_pool.tile([P, GMAX], F32, tag="t2")
                # y0 bits = 0x5f3759df - (bits(v) >> 1)
                nc.vector.tensor_scalar(
                    out=rg.bitcast(I32)[:, 0:g], in0=vd.bitcast(I32)[:, 0:g],
                    scalar1=1, scalar2=0xFFFFFFFF,
                    op0=ALU.logical_shift_right, op1=ALU.bitwise_xor)
                nc.vector.tensor_scalar(
                    out=rg.bitcast(I32)[:, 0:g], in0=rg.bitcast(I32)[:, 0:g],
                    scalar1=0x5F3759E0, scalar2=None, op0=ALU.add)
                for _ in range(2):  # y *= 1.5 - 0.5*v*y*y  (~1e-5 rel err)
                    nc.vector.tensor_mul(out=t1[:, 0:g], in0=rg[:, 0:g], in1=rg[:, 0:g])
                    nc.vector.tensor_mul(out=t2[:, 0:g], in0=t1[:, 0:g], in1=vd[:, 0:g])
                    nc.vector.tensor_scalar(
                        out=t2[:, 0:g], in0=t2[:, 0:g],
                        scalar1=-0.5, scalar2=1.5, op0=ALU.mult, op1=ALU.add)
                    nc.vector.tensor_mul(out=rg[:, 0:g], in0=rg[:, 0:g], in1=t2[:, 0:g])
                return rg

            # --- main loop: software-pipelined groups --------------------------
            # Group g+1's DMA + bn_stats interleave with group g's adapter math
            # so neither DVE nor PE ever drains at a group boundary.
            x_tiles = {}
            pending = []  # (tile_idx, mean_ap, rstd_ap) with stats ready
            base = 0
            for g in GROUPS:
                mvg = gstat_pool.tile([P, GMAX, 2], F32, tag="mvg")
                for j in range(g):
                    i = base + j
                    x_sb = xin_pool.tile([P, H], F32, tag="x")
                    x_tiles[i] = x_sb
                    nc.sync.dma_start(out=x_sb, in_=x_rows[i])
                    st = bns_pool.tile([P, 2, 6], F32, tag="bns")
                    nc.vector.bn_stats(st[:, 0, :], x_sb[:, 0:512])
                    nc.vector.bn_stats(st[:, 1, :], x_sb[:, 512:1024])
                    nc.vector.bn_aggr(mvg[:, j, :], st)
                    if pending:
                        process_tile(*pending.pop(0))
                rg = group_rstd(mvg, g)
                pending.extend(
                    (base + j, mvg[:, j, 0:1], rg[:, j:j + 1]) for j in range(g))
                base += g
            for args in pending:
                process_tile(*args)

    return nc


_CACHE: dict = {}


def _get_nc() -> bass.Bass:
    if "nc" not in _CACHE:
        nc = build_kernel()
        nc.finalize()
        _CACHE["nc"] = nc
    return _CACHE["nc"]


def make_in_maps(hidden_states, ln_gamma, ln_beta, w_down, b_down, w_up, b_up):
    x = np.ascontiguousarray(np.asarray(hidden_states, dtype=np.float32))
    g = np.asarray(ln_gamma, dtype=np.float32)
    be = np.asarray(ln_beta, dtype=np.float32)
    wd = np.asarray(w_down, dtype=np.float32)
    bd = np.asarray(b_down, dtype=np.float32)
    wu = np.asarray(w_up, dtype=np.float32)
    bu = np.asarray(b_up, dtype=np.float32)

    # Fold LN affine into the down projection:
    #   (xhat*g + be) @ wd + bd == xhat @ (g[:,None]*wd) + (be @ wd + bd)
    wd_eff = np.ascontiguousarray((g[:, None] * wd).astype(ml_dtypes.bfloat16))
    bd_eff = np.ascontiguousarray((bd + be @ wd).reshape(R, 1).astype(np.float32))
    # Fold b_up into the up matmul via an appended ones-row on the left operand.
    wua = np.ascontiguousarray(
        np.concatenate([wu, bu[None, :]], axis=0).astype(ml_dtypes.bfloat16))

    x_shards = x.reshape(N_CORES, TOK, H)
    return [
        {
            "hidden_states": np.ascontiguousarray(x_shards[c]),
            "w_down": wd_eff,
            "b_down": bd_eff,
            "w_up_aug": wua,
        }
        for c in range(N_CORES)
    ]


def run_device(in_maps, **kwargs):
    nc = _get_nc()
    return run_bass_kernel_spmd(nc, in_maps, core_ids=list(range(N_CORES)), **kwargs)


def kernel(hidden_states, ln_gamma, ln_beta, w_down, b_down, w_up, b_up):
    in_maps = make_in_maps(hidden_states, ln_gamma, ln_beta,
                           w_down, b_down, w_up, b_up)
    res = run_device(in_maps)
    out = np.stack([res.results[c]["out"] for c in range(N_CORES)], axis=0)
    return np.ascontiguousarray(out.reshape(B, S, H).astype(np.float32, copy=False))
